# revision 2
# baseline (speedup 1.0000x reference)
"""Trainium2 Bass kernel for the per-game CriticNetwork (MoE-routed MLP).

Network (per sample b, with game g = idx[b]):
    h1  = relu(W1[g] @ state[b] + b1[g])          # [600]
    h2  = W2s @ h1 + b2s + W2a[g] @ action[b]     # [500]
    q   = W3[g] . relu(h2) + b3[g]                # scalar

v2: fp8 (float8e4) DoubleRow matmuls. The cost model runs a DoubleRow
matmul (two K<=128 tiles contracted per instruction) at 0.5 cycles/row,
so one DR instruction does 2 K-chunks in 1/4 the bf16 PE time.

Precision plan (rel err ~1.4e-2 vs 2e-2 budget, verified in numpy):
  - state is split hi/lo into two fp8 k-tiles (exact to ~fp16), W1 plain fp8
  - h1 is written as plain fp8 by the ACT relu (scale folded into ACT)
  - W2s plain fp8; the action path (dominant error source) is 3-term
    compensated: A*a ~= Ahi*ahi + Ahi*alo + Alo*ahi (all fp8 hi/lo splits)
  - L3 (q head) stays bf16: 4 plain matmuls, W3/hf never in fp8
All scales are powers of two; psum-scale consistency lets the hf relu run
as a 2-op tensor_scalar on DVE/Pool (bias pre-scaled, no multiplier).

Per tile t (512 samples, single game, PE work 21 DR + 4 bf16 = ~3.1us):
  L1: 5 DR  (xhi,xlo) pairs vs (W1c,W1c)          -> ps1, ACT relu -> fp8 h1
  L2: per m: DR(c0,c1)+DR(c2,c3) from shared W2s, DR(hhi4,ahi) vs
      (W2s4,Ahi), DR(ahi,alo) vs (Alo,Ahi)        -> ps2, DVE/Pool relu -> bf16 hf
  L3: 4 bf16 matmuls K=128,M=1 -> q at psum partition 32*(t%4)
Software pipeline (PE stream): [L1_t | L2_{t-1} | L3_{t-2}] so the PE never
waits on the ACT relu chain.  q: one engine copy per 4-tile psum group to
SBUF, one output DMA at the end.  All activations and weights resident in
SBUF; ~15 DMAs per core total.
"""

import numpy as np
import ml_dtypes

import concourse.bass as bass
import concourse.mybir as mybir
import concourse.tile as tile
from concourse import bacc
from concourse.bass import ts
from concourse.bass_utils import run_bass_kernel_spmd

F32 = mybir.dt.float32
BF16 = mybir.dt.bfloat16
F8 = mybir.dt.float8e4
DR = mybir.MatmulPerfMode.DoubleRow
RELU = mybir.ActivationFunctionType.Relu
NP_F8 = mybir.dt.np(F8)          # ml_dtypes.float8_e4m3
NP_BF16 = mybir.dt.np(BF16)

G = 8          # games
D = 128        # state dim
A = 16         # action dim
H1 = 600       # hidden 1 (padded to 640 = 5 * 128)
H2 = 500       # hidden 2 (padded to 512 = 4 * 128)
B = 32768      # batch
H1P, H2P = 640, 512
K1 = H1P // 128   # 5 h1 chunks
M2 = H2P // 128   # 4 h2 chunks
T = 512        # samples per tile (one PSUM bank of fp32)
NCORES = 8
NT = 9         # tiles per core; 72 total >= 64 + 7 worst-case segment padding
BPC = NT * T   # 4608 lanes per core
NQ = (NT + 3) // 4   # psum q groups (tiles per group: 4 at partitions 0/32/64/96)

# Power-of-two scales keep every fp8 operand in e4m3 range (max 240) and
# make the two psum paths of L2 scale-consistent: SH1*SW2S == SA*SW2A.
SX, SW1, SH1, SW2S, SA, SW2A = 32.0, 2048.0, 32.0, 512.0, 32.0, 512.0
S1 = SX * SW1          # L1 psum scale (65536)
S2 = SH1 * SW2S        # L2 psum scale (16384) == SA*SW2A
K1SCALE = SH1 / S1     # ACT h1 relu scale (2^-11)

WB_W1 = 2 * K1 * 128       # 1280: chunk c at cols 256c as (W1c | W1c)
WB_ST = 4 * 512            # (W2s4 | Ahi | Alo | Ahi)
WBLOB = WB_W1 + WB_ST      # 3328

CFG = {
    "hf_eng": ("vector", "vector", "vector", "vector"),  # per-m hf relu engine
    "q_eng": "vector",
    "ps1_bufs": 3,
    "ps2_bufs": 4,
}

_NC = None


def build_nc():
    nc = bacc.Bacc("TRN2", target_bir_lowering=False, debug=False,
                   num_devices=NCORES)

    stateHL = nc.declare_dram_parameter("stateHL", [128, 2, BPC], F8,
                                        isOutput=False)
    actHL = nc.declare_dram_parameter("actHL", [128, 2, BPC], F8,
                                      isOutput=False)
    wblob = nc.declare_dram_parameter("wblob", [NT, 128, WBLOB], F8,
                                      isOutput=False)
    w2st = nc.declare_dram_parameter("w2st", [128, 4 * 512], F8,
                                     isOutput=False)
    w3t = nc.declare_dram_parameter("w3t", [128, NT * M2], BF16,
                                    isOutput=False)
    b1t = nc.declare_dram_parameter("b1t", [128, NT * K1], F32, isOutput=False)
    b2st = nc.declare_dram_parameter("b2st", [128, M2], F32, isOutput=False)
    qdr = nc.declare_dram_parameter("q", [1, NT * T], F32, isOutput=True)

    with tile.TileContext(nc) as tc:
        with (
            tc.tile_pool(name="const", bufs=1) as const,
            tc.tile_pool(name="wts", bufs=3) as wts,
            tc.tile_pool(name="ps1", bufs=CFG["ps1_bufs"], space="PSUM") as ps1p,
            tc.tile_pool(name="ps2", bufs=CFG["ps2_bufs"], space="PSUM") as ps2p,
            tc.tile_pool(name="psq", bufs=1, space="PSUM") as psqp,
        ):
            state_sb = const.tile([128, 2, BPC], F8)
            movb = const.tile([128, 7, BPC], F8)   # 0-4: h1 chunks, 5: ahi, 6: alo
            hfb = const.tile([128, M2, BPC], BF16)
            w2s_sb = const.tile([128, 4, 512], F8)
            w3_sb = const.tile([128, NT * M2], BF16)
            b1_sb = const.tile([128, NT * K1], F32)
            b2_sb = const.tile([128, M2], F32)
            qsb = const.tile([1, NT * T], F32)

            def load_startup():
                # ordered by first use; tile-0 slices first so compute can start
                nc.sync.dma_start(state_sb[:, :, 0:T], stateHL.ap()[:, :, 0:T])
                nc.sync.dma_start(b1_sb[:], b1t.ap())
                nc.sync.dma_start(state_sb[:, :, T:BPC], stateHL.ap()[:, :, T:BPC])
                nc.sync.dma_start(w2s_sb[:],
                                  w2st.ap().rearrange("p (c n) -> p c n", c=4))
                nc.sync.dma_start(movb[:, 5:7, :], actHL.ap())
                nc.sync.dma_start(w3_sb[:], w3t.ap())
                nc.sync.dma_start(b2_sb[:], b2st.ap())

            def emit_l1(t, wb):
                cols = ts(t, T)
                for c in range(K1):
                    ps1 = ps1p.tile([128, T], F32, tag="ps1")
                    lhsT = wb[:, 256 * c:256 * (c + 1)].rearrange(
                        "p (j m) -> p j m", j=2)
                    nc.tensor.matmul(ps1[:], lhsT, state_sb[:, :, cols],
                                     start=True, stop=True, perf_mode=DR)
                    nc.scalar.activation(
                        movb[:, c, cols], ps1[:], RELU,
                        bias=b1_sb[:, t * K1 + c:t * K1 + c + 1],
                        scale=K1SCALE)

            def relu_hf(m, ps2, cols):
                eng = getattr(nc, CFG["hf_eng"][m])
                eng.tensor_scalar(hfb[:, m, cols], ps2[:],
                                  b2_sb[:, m:m + 1], 0.0,
                                  mybir.AluOpType.add, mybir.AluOpType.max)

            def emit_l2(t, wb):
                cols = ts(t, T)
                st = wb[:, WB_W1:].rearrange("p (s n) -> p s n", s=4)
                ps2s = []
                # DR12 block: shared-W2s chunks 0-3 (needs only h1 c0..c3)
                for m in range(M2):
                    ps2 = ps2p.tile([128, T], F32, tag="ps2")
                    for cc in (0, 2):
                        nc.tensor.matmul(ps2[:],
                                         w2s_sb[:, cc:cc + 2, ts(m, 128)],
                                         movb[:, cc:cc + 2, cols],
                                         start=(cc == 0), stop=False,
                                         perf_mode=DR)
                    ps2s.append(ps2)
                # DR34 block: (W2s4|Ahi)x(hhi4,ahi) and (Alo|Ahi)x(ahi,alo)
                for m in range(M2):
                    nc.tensor.matmul(ps2s[m][:], st[:, 0:2, ts(m, 128)],
                                     movb[:, 4:6, cols], start=False,
                                     stop=False, perf_mode=DR)
                    nc.tensor.matmul(ps2s[m][:], st[:, 2:4, ts(m, 128)],
                                     movb[:, 5:7, cols], start=False,
                                     stop=True, perf_mode=DR)
                    relu_hf(m, ps2s[m], cols)

            def emit_l3(t):
                cols = ts(t, T)
                psq = psqp.tile([1, T], F32, tag="psq")
                for m in range(M2):
                    nc.tensor.matmul(psq[:],
                                     w3_sb[:, t * M2 + m:t * M2 + m + 1],
                                     hfb[:, m, cols],
                                     start=(m == 0), stop=(m == M2 - 1))
                eng = nc.vector if t % 2 == 0 else nc.scalar
                if t % 2 == 0:
                    eng.tensor_copy(qsb[:, cols], psq[:])
                else:
                    eng.activation(qsb[:, cols], psq[:],
                                   mybir.ActivationFunctionType.Copy)

            pend = []   # [(t, wb)] pipeline: L2 lags 1 tile, L3 lags 2
            for t in range(NT):
                wb = wts.tile([128, WBLOB], F8, tag="wb")
                nc.sync.dma_start(wb[:], wblob[t])
                if t == 0:
                    load_startup()
                emit_l1(t, wb)
                if len(pend) >= 1:
                    emit_l2(*pend[-1])
                if len(pend) >= 2:
                    emit_l3(pend[-2][0])
                    pend.pop(0)
                pend.append((t, wb))
            emit_l2(*pend[-1])
            emit_l3(pend[-2][0])
            emit_l3(pend[-1][0])
            nc.sync.dma_start(qdr.ap(), qsb[:])

    nc.compile()
    return nc


def _get_nc():
    global _NC
    if _NC is None:
        _NC = build_nc()
    return _NC


def _f8(x):
    return np.asarray(x, np.float32).astype(NP_F8)


def _plan_tiles(idx):
    """Stable-sort samples by game, pad each game segment to 512-sample
    tiles, pad the tile list to the fixed 72. Returns (sel, valid, gids):
    sel[t, l] = original sample index feeding lane l of tile t."""
    perm = np.argsort(idx, kind="stable")
    counts = np.bincount(idx, minlength=G)
    ntot = NCORES * NT
    sel = np.zeros((ntot, T), np.int64)
    valid = np.zeros((ntot, T), bool)
    gids = np.zeros(ntot, np.int64)
    pos, t = 0, 0
    for g in range(G):
        cg = int(counts[g])
        for k in range((cg + T - 1) // T):
            n = min(T, cg - k * T)
            lanes = perm[pos:pos + n]
            sel[t, :n] = lanes
            valid[t, :n] = True
            if n < T:
                sel[t, n:] = lanes[0]
            gids[t] = g
            pos += n
            t += 1
    assert t <= ntot, f"tile plan overflow: {t} > {ntot}"
    return sel, valid, gids


def build_in_maps(inputs):
    state = np.ascontiguousarray(np.asarray(inputs["state"], np.float32))
    action = np.ascontiguousarray(np.asarray(inputs["action"], np.float32))
    idx = np.asarray(inputs["idx"]).astype(np.int64)
    W1 = np.asarray(inputs["W1"], np.float32)
    b1 = np.asarray(inputs["b1"], np.float32)
    W2s = np.asarray(inputs["W2s"], np.float32)
    b2s = np.asarray(inputs["b2s"], np.float32)
    W2a = np.asarray(inputs["W2a"], np.float32)
    W3 = np.asarray(inputs["W3"], np.float32)
    assert state.shape == (B, D) and action.shape == (B, A)

    sel, valid, gids = _plan_tiles(idx)

    # --- game-indexed weight blocks (host-quantized, fp8 scaled) ---
    # W1 pairs: [G, 128, 1280] with chunk c at cols 256c as (W1c | W1c)
    W1T = np.zeros((G, D, H1P), np.float32)
    W1T[:, :, :H1] = W1.transpose(0, 2, 1) * SW1
    W1q = _f8(W1T)
    w1blob = np.zeros((G, 128, WB_W1), NP_F8)
    for c in range(K1):
        blk = W1q[:, :, 128 * c:128 * (c + 1)]
        w1blob[:, :, 256 * c:256 * c + 128] = blk
        w1blob[:, :, 256 * c + 128:256 * c + 256] = blk

    # shared W2s chunks 0-3 + per-game ST blob (W2s4 | Ahi | Alo | Ahi)
    W2sT = np.zeros((H1P, H2P), np.float32)
    W2sT[:H1, :H2] = W2s.T * SW2S
    W2sq = _f8(W2sT)
    w2st_np = np.ascontiguousarray(
        W2sq[:512].reshape(4, 128, H2P).transpose(1, 0, 2).reshape(128, 4 * 512))

    W2aT = np.zeros((G, 128, H2P), np.float32)
    W2aT[:, :A, :H2] = W2a.transpose(0, 2, 1) * SW2A
    Ahi = _f8(W2aT)
    Alo = _f8(W2aT - Ahi.astype(np.float32))
    stblob = np.zeros((G, 128, WB_ST), NP_F8)
    stblob[:, :, 0:512] = np.broadcast_to(W2sq[512:640], (G, 128, 512))
    stblob[:, :, 512:1024] = Ahi
    stblob[:, :, 1024:1536] = Alo
    stblob[:, :, 1536:2048] = Ahi

    W3P = np.zeros((G, H2P), np.float32)
    W3P[:, :H2] = W3
    W3T = np.ascontiguousarray(
        W3P.reshape(G, M2, 128).transpose(0, 2, 1)).astype(NP_BF16)  # [G,128,4]

    b1P = np.zeros((G, H1P), np.float32)
    b1P[:, :H1] = b1 * SH1
    b1T = np.ascontiguousarray(b1P.reshape(G, K1, 128).transpose(0, 2, 1))

    b2sP = np.zeros(H2P, np.float32)
    b2sP[:H2] = b2s * S2
    b2st_np = np.ascontiguousarray(b2sP.reshape(M2, 128).T)

    in_maps = []
    for core in range(NCORES):
        tsl = slice(core * NT, (core + 1) * NT)
        lanes = sel[tsl].reshape(-1)
        gt = gids[tsl]

        xs = state[lanes].T * SX                  # [128, BPC]
        xhi = _f8(xs)
        xlo = _f8(xs - xhi.astype(np.float32))
        stateHL = np.stack([xhi, xlo], axis=1)    # [128, 2, BPC]

        acts = action[lanes].T * SA               # [16, BPC]
        ahi = _f8(acts)
        alo = _f8(acts - ahi.astype(np.float32))
        actHL = np.zeros((128, 2, BPC), NP_F8)
        actHL[:A, 0, :] = ahi
        actHL[:A, 1, :] = alo

        wblob_np = np.concatenate([w1blob[gt], stblob[gt]], axis=2)

        w3t_np = np.ascontiguousarray(
            W3T[gt].transpose(1, 0, 2).reshape(128, NT * M2))
        b1t_np = np.ascontiguousarray(
            b1T[gt].transpose(1, 0, 2).reshape(128, NT * K1))

        in_maps.append({
            "stateHL": np.ascontiguousarray(stateHL),
            "actHL": actHL,
            "wblob": np.ascontiguousarray(wblob_np),
            "w2st": w2st_np,
            "w3t": w3t_np,
            "b1t": b1t_np,
            "b2st": b2st_np,
        })
    return in_maps, sel, valid


def kernel(**inputs):
    idx = np.asarray(inputs["idx"]).astype(np.int64)
    b3 = np.asarray(inputs["b3"], np.float32)
    in_maps, sel, valid = build_in_maps(inputs)

    res = run_bass_kernel_spmd(_get_nc(), in_maps, list(range(NCORES))).results
    qv = np.concatenate([np.asarray(res[core]["q"], np.float32).reshape(-1)
                         for core in range(NCORES)])
    qv /= S2

    out = np.zeros(B, np.float32)
    flat_sel = sel.reshape(-1)
    flat_valid = valid.reshape(-1)
    out[flat_sel[flat_valid]] = qv[flat_valid]
    out += b3[idx]
    return out.astype(np.float32)


# revision 4
# speedup vs baseline: 1.0905x; 1.0905x over previous
"""Trainium2 Bass kernel for the per-game CriticNetwork (MoE-routed MLP).

Network (per sample b, with game g = idx[b]):
    h1  = relu(W1[g] @ state[b] + b1[g])          # [600]
    h2  = W2s @ h1 + b2s + W2a[g] @ action[b]     # [500]
    q   = W3[g] . relu(h2) + b3[g]                # scalar

v2: fp8 (float8e4) DoubleRow matmuls. The cost model runs a DoubleRow
matmul (two K<=128 tiles contracted per instruction) at 0.5 cycles/row,
so one DR instruction does 2 K-chunks in 1/4 the bf16 PE time.

Precision plan (rel err ~1.4e-2 vs 2e-2 budget, verified in numpy):
  - state is split hi/lo into two fp8 k-tiles (exact to ~fp16), W1 plain fp8
  - h1 is written as plain fp8 by the ACT relu (scale folded into ACT)
  - W2s plain fp8; the action path (dominant error source) is 3-term
    compensated: A*a ~= Ahi*ahi + Ahi*alo + Alo*ahi (all fp8 hi/lo splits)
  - L3 (q head) stays bf16: 4 plain matmuls, W3/hf never in fp8
All scales are powers of two; psum-scale consistency lets the hf relu run
as a 2-op tensor_scalar on DVE/Pool (bias pre-scaled, no multiplier).

Per tile t (512 samples, single game, PE work 21 DR + 4 bf16 = ~3.1us):
  L1: 5 DR  (xhi,xlo) pairs vs (W1c,W1c)          -> ps1, ACT relu -> fp8 h1
  L2: per m: DR(c0,c1)+DR(c2,c3) from shared W2s, DR(hhi4,ahi) vs
      (W2s4,Ahi), DR(ahi,alo) vs (Alo,Ahi)        -> ps2, DVE/Pool relu -> bf16 hf
  L3: 4 bf16 matmuls K=128,M=1 -> q at psum partition 32*(t%4)
Software pipeline (PE stream): [L1_t | L2_{t-1} | L3_{t-2}] so the PE never
waits on the ACT relu chain.  q: one engine copy per 4-tile psum group to
SBUF, one output DMA at the end.  All activations and weights resident in
SBUF; ~15 DMAs per core total.
"""

import numpy as np
import ml_dtypes

import concourse.bass as bass
import concourse.mybir as mybir
import concourse.tile as tile
from concourse import bacc
from concourse.bass import ts
from concourse.bass_utils import run_bass_kernel_spmd

F32 = mybir.dt.float32
BF16 = mybir.dt.bfloat16
F8 = mybir.dt.float8e4
DR = mybir.MatmulPerfMode.DoubleRow
RELU = mybir.ActivationFunctionType.Relu
NP_F8 = mybir.dt.np(F8)          # ml_dtypes.float8_e4m3
NP_BF16 = mybir.dt.np(BF16)

G = 8          # games
D = 128        # state dim
A = 16         # action dim
H1 = 600       # hidden 1 (padded to 640 = 5 * 128)
H2 = 500       # hidden 2 (padded to 512 = 4 * 128)
B = 32768      # batch
H1P, H2P = 640, 512
K1 = H1P // 128   # 5 h1 chunks
M2 = H2P // 128   # 4 h2 chunks
T = 512        # samples per tile (one PSUM bank of fp32)
NCORES = 8
NT = 9         # tiles per core; 72 total >= 64 + 7 worst-case segment padding
BPC = NT * T   # 4608 lanes per core
NQ = (NT + 3) // 4   # psum q groups (tiles per group: 4 at partitions 0/32/64/96)

# Power-of-two scales keep every fp8 operand in e4m3 range (max 240) and
# make the two psum paths of L2 scale-consistent: SH1*SW2S == SA*SW2A.
SX, SW1, SH1, SW2S, SA, SW2A = 32.0, 2048.0, 32.0, 512.0, 32.0, 512.0
S1 = SX * SW1          # L1 psum scale (65536)
S2 = SH1 * SW2S        # L2 psum scale (16384) == SA*SW2A
K1SCALE = SH1 / S1     # ACT h1 relu scale (2^-11)

WB_W1 = 2 * K1 * 128       # 1280: chunk c at cols 256c as (W1c | W1c)
WB_ST = 4 * 512            # (W2s4 | Ahi | Alo | Ahi)
WBLOB = WB_W1 + WB_ST      # 3328

CFG = {
    "hf_eng": ("vector", "vector", "vector", "vector"),  # per-m hf relu engine
    "q_eng": "vector",
    "ps1_bufs": 3,
    "ps2_bufs": 4,
}

_NC = None


def build_nc():
    nc = bacc.Bacc("TRN2", target_bir_lowering=False, debug=False,
                   num_devices=NCORES)

    stateHL = nc.declare_dram_parameter("stateHL", [128, 2, BPC], F8,
                                        isOutput=False)
    actHL = nc.declare_dram_parameter("actHL", [128, 2, BPC], F8,
                                      isOutput=False)
    wblob = nc.declare_dram_parameter("wblob", [NT, 128, WBLOB], F8,
                                      isOutput=False)
    w2st = nc.declare_dram_parameter("w2st", [128, 4 * 512], F8,
                                     isOutput=False)
    w3t = nc.declare_dram_parameter("w3t", [128, NT * M2], BF16,
                                    isOutput=False)
    b1t = nc.declare_dram_parameter("b1t", [128, NT * K1], F32, isOutput=False)
    b2st = nc.declare_dram_parameter("b2st", [128, M2], F32, isOutput=False)
    qdr = nc.declare_dram_parameter("q", [1, NT * T], F32, isOutput=True)

    with tile.TileContext(nc) as tc:
        with (
            tc.tile_pool(name="const", bufs=1) as const,
            tc.tile_pool(name="wts", bufs=3) as wts,
            tc.tile_pool(name="ps1", bufs=CFG["ps1_bufs"], space="PSUM") as ps1p,
            tc.tile_pool(name="ps2", bufs=CFG["ps2_bufs"], space="PSUM") as ps2p,
            tc.tile_pool(name="psq", bufs=1, space="PSUM") as psqp,
        ):
            state_sb = const.tile([128, 2, BPC], F8)
            movb = const.tile([128, 7, BPC], F8)   # 0-4: h1 chunks, 5: ahi, 6: alo
            hfb = const.tile([128, M2, BPC], BF16)
            w2s_sb = const.tile([128, 4, 512], F8)
            w3_sb = const.tile([128, NT * M2], BF16)
            b1_sb = const.tile([128, NT * K1], F32)
            b2_sb = const.tile([128, M2], F32)
            qsb = const.tile([1, NT * T], F32)

            def load_startup(t):
                # staged just-in-time so early tiles aren't gated on the big
                # resident loads (the DMA device drains in issue order)
                if t == 0:
                    nc.sync.dma_start(state_sb[:, :, 0:2 * T],
                                      stateHL.ap()[:, :, 0:2 * T])
                    nc.sync.dma_start(b1_sb[:], b1t.ap())
                elif t == 1:
                    nc.sync.dma_start(movb[:, 5:7, 0:2 * T],
                                      actHL.ap()[:, :, 0:2 * T])
                    nc.sync.dma_start(w2s_sb[:],
                                      w2st.ap().rearrange("p (c n) -> p c n",
                                                          c=4))
                    nc.sync.dma_start(b2_sb[:], b2st.ap())
                    nc.sync.dma_start(w3_sb[:], w3t.ap())
                elif t == 2:
                    nc.sync.dma_start(state_sb[:, :, 2 * T:5 * T],
                                      stateHL.ap()[:, :, 2 * T:5 * T])
                    nc.sync.dma_start(movb[:, 5:7, 2 * T:4 * T],
                                      actHL.ap()[:, :, 2 * T:4 * T])
                elif t == 3:
                    nc.sync.dma_start(state_sb[:, :, 5 * T:BPC],
                                      stateHL.ap()[:, :, 5 * T:BPC])
                    nc.sync.dma_start(movb[:, 5:7, 4 * T:BPC],
                                      actHL.ap()[:, :, 4 * T:BPC])

            def emit_l1(t, wb):
                cols = ts(t, T)
                for c in range(K1):
                    ps1 = ps1p.tile([128, T], F32, tag="ps1")
                    lhsT = wb[:, 256 * c:256 * (c + 1)].rearrange(
                        "p (j m) -> p j m", j=2)
                    nc.tensor.matmul(ps1[:], lhsT, state_sb[:, :, cols],
                                     start=True, stop=True, perf_mode=DR)
                    nc.scalar.activation(
                        movb[:, c, cols], ps1[:], RELU,
                        bias=b1_sb[:, t * K1 + c:t * K1 + c + 1],
                        scale=K1SCALE)

            def relu_hf(m, ps2, cols):
                eng = getattr(nc, CFG["hf_eng"][m])
                eng.tensor_scalar(hfb[:, m, cols], ps2[:],
                                  b2_sb[:, m:m + 1], 0.0,
                                  mybir.AluOpType.add, mybir.AluOpType.max)

            def emit_l2(t, wb):
                cols = ts(t, T)
                st = wb[:, WB_W1:].rearrange("p (s n) -> p s n", s=4)
                ps2s = []
                # DR12 block: shared-W2s chunks 0-3 (needs only h1 c0..c3)
                for m in range(M2):
                    ps2 = ps2p.tile([128, T], F32, tag="ps2")
                    for cc in (0, 2):
                        nc.tensor.matmul(ps2[:],
                                         w2s_sb[:, cc:cc + 2, ts(m, 128)],
                                         movb[:, cc:cc + 2, cols],
                                         start=(cc == 0), stop=False,
                                         perf_mode=DR)
                    ps2s.append(ps2)
                # DR34 block: (W2s4|Ahi)x(hhi4,ahi) and (Alo|Ahi)x(ahi,alo)
                for m in range(M2):
                    nc.tensor.matmul(ps2s[m][:], st[:, 0:2, ts(m, 128)],
                                     movb[:, 4:6, cols], start=False,
                                     stop=False, perf_mode=DR)
                    nc.tensor.matmul(ps2s[m][:], st[:, 2:4, ts(m, 128)],
                                     movb[:, 5:7, cols], start=False,
                                     stop=True, perf_mode=DR)
                    relu_hf(m, ps2s[m], cols)

            def emit_l3(t):
                cols = ts(t, T)
                psq = psqp.tile([1, T], F32, tag="psq")
                for m in range(M2):
                    nc.tensor.matmul(psq[:],
                                     w3_sb[:, t * M2 + m:t * M2 + m + 1],
                                     hfb[:, m, cols],
                                     start=(m == 0), stop=(m == M2 - 1))
                eng = nc.vector if t % 2 == 0 else nc.scalar
                if t % 2 == 0:
                    eng.tensor_copy(qsb[:, cols], psq[:])
                else:
                    eng.activation(qsb[:, cols], psq[:],
                                   mybir.ActivationFunctionType.Copy)

            pend = []   # [(t, wb)] pipeline: L2 lags 1 tile, L3 lags 2
            for t in range(NT):
                wb = wts.tile([128, WBLOB], F8, tag="wb")
                nc.sync.dma_start(wb[:], wblob[t])
                load_startup(t)
                emit_l1(t, wb)
                if len(pend) >= 1:
                    emit_l2(*pend[-1])
                if len(pend) >= 2:
                    emit_l3(pend[-2][0])
                    pend.pop(0)
                pend.append((t, wb))
            emit_l2(*pend[-1])
            emit_l3(pend[-2][0])
            emit_l3(pend[-1][0])
            nc.sync.dma_start(qdr.ap(), qsb[:])

    nc.compile()
    return nc


def _get_nc():
    global _NC
    if _NC is None:
        _NC = build_nc()
    return _NC


def _f8(x):
    return np.asarray(x, np.float32).astype(NP_F8)


def _plan_tiles(idx):
    """Stable-sort samples by game, pad each game segment to 512-sample
    tiles, pad the tile list to the fixed 72. Returns (sel, valid, gids):
    sel[t, l] = original sample index feeding lane l of tile t."""
    perm = np.argsort(idx, kind="stable")
    counts = np.bincount(idx, minlength=G)
    ntot = NCORES * NT
    sel = np.zeros((ntot, T), np.int64)
    valid = np.zeros((ntot, T), bool)
    gids = np.zeros(ntot, np.int64)
    pos, t = 0, 0
    for g in range(G):
        cg = int(counts[g])
        for k in range((cg + T - 1) // T):
            n = min(T, cg - k * T)
            lanes = perm[pos:pos + n]
            sel[t, :n] = lanes
            valid[t, :n] = True
            if n < T:
                sel[t, n:] = lanes[0]
            gids[t] = g
            pos += n
            t += 1
    assert t <= ntot, f"tile plan overflow: {t} > {ntot}"
    return sel, valid, gids


def build_in_maps(inputs):
    state = np.ascontiguousarray(np.asarray(inputs["state"], np.float32))
    action = np.ascontiguousarray(np.asarray(inputs["action"], np.float32))
    idx = np.asarray(inputs["idx"]).astype(np.int64)
    W1 = np.asarray(inputs["W1"], np.float32)
    b1 = np.asarray(inputs["b1"], np.float32)
    W2s = np.asarray(inputs["W2s"], np.float32)
    b2s = np.asarray(inputs["b2s"], np.float32)
    W2a = np.asarray(inputs["W2a"], np.float32)
    W3 = np.asarray(inputs["W3"], np.float32)
    assert state.shape == (B, D) and action.shape == (B, A)

    sel, valid, gids = _plan_tiles(idx)

    # --- game-indexed weight blocks (host-quantized, fp8 scaled) ---
    # W1 pairs: [G, 128, 1280] with chunk c at cols 256c as (W1c | W1c)
    W1T = np.zeros((G, D, H1P), np.float32)
    W1T[:, :, :H1] = W1.transpose(0, 2, 1) * SW1
    W1q = _f8(W1T)
    w1blob = np.zeros((G, 128, WB_W1), NP_F8)
    for c in range(K1):
        blk = W1q[:, :, 128 * c:128 * (c + 1)]
        w1blob[:, :, 256 * c:256 * c + 128] = blk
        w1blob[:, :, 256 * c + 128:256 * c + 256] = blk

    # shared W2s chunks 0-3 + per-game ST blob (W2s4 | Ahi | Alo | Ahi)
    W2sT = np.zeros((H1P, H2P), np.float32)
    W2sT[:H1, :H2] = W2s.T * SW2S
    W2sq = _f8(W2sT)
    w2st_np = np.ascontiguousarray(
        W2sq[:512].reshape(4, 128, H2P).transpose(1, 0, 2).reshape(128, 4 * 512))

    W2aT = np.zeros((G, 128, H2P), np.float32)
    W2aT[:, :A, :H2] = W2a.transpose(0, 2, 1) * SW2A
    Ahi = _f8(W2aT)
    Alo = _f8(W2aT - Ahi.astype(np.float32))
    stblob = np.zeros((G, 128, WB_ST), NP_F8)
    stblob[:, :, 0:512] = np.broadcast_to(W2sq[512:640], (G, 128, 512))
    stblob[:, :, 512:1024] = Ahi
    stblob[:, :, 1024:1536] = Alo
    stblob[:, :, 1536:2048] = Ahi

    W3P = np.zeros((G, H2P), np.float32)
    W3P[:, :H2] = W3
    W3T = np.ascontiguousarray(
        W3P.reshape(G, M2, 128).transpose(0, 2, 1)).astype(NP_BF16)  # [G,128,4]

    b1P = np.zeros((G, H1P), np.float32)
    b1P[:, :H1] = b1 * SH1
    b1T = np.ascontiguousarray(b1P.reshape(G, K1, 128).transpose(0, 2, 1))

    b2sP = np.zeros(H2P, np.float32)
    b2sP[:H2] = b2s * S2
    b2st_np = np.ascontiguousarray(b2sP.reshape(M2, 128).T)

    in_maps = []
    for core in range(NCORES):
        tsl = slice(core * NT, (core + 1) * NT)
        lanes = sel[tsl].reshape(-1)
        gt = gids[tsl]

        xs = state[lanes].T * SX                  # [128, BPC]
        xhi = _f8(xs)
        xlo = _f8(xs - xhi.astype(np.float32))
        stateHL = np.stack([xhi, xlo], axis=1)    # [128, 2, BPC]

        acts = action[lanes].T * SA               # [16, BPC]
        ahi = _f8(acts)
        alo = _f8(acts - ahi.astype(np.float32))
        actHL = np.zeros((128, 2, BPC), NP_F8)
        actHL[:A, 0, :] = ahi
        actHL[:A, 1, :] = alo

        wblob_np = np.concatenate([w1blob[gt], stblob[gt]], axis=2)

        w3t_np = np.ascontiguousarray(
            W3T[gt].transpose(1, 0, 2).reshape(128, NT * M2))
        b1t_np = np.ascontiguousarray(
            b1T[gt].transpose(1, 0, 2).reshape(128, NT * K1))

        in_maps.append({
            "stateHL": np.ascontiguousarray(stateHL),
            "actHL": actHL,
            "wblob": np.ascontiguousarray(wblob_np),
            "w2st": w2st_np,
            "w3t": w3t_np,
            "b1t": b1t_np,
            "b2st": b2st_np,
        })
    return in_maps, sel, valid


def kernel(**inputs):
    idx = np.asarray(inputs["idx"]).astype(np.int64)
    b3 = np.asarray(inputs["b3"], np.float32)
    in_maps, sel, valid = build_in_maps(inputs)

    res = run_bass_kernel_spmd(_get_nc(), in_maps, list(range(NCORES))).results
    qv = np.concatenate([np.asarray(res[core]["q"], np.float32).reshape(-1)
                         for core in range(NCORES)])
    qv /= S2

    out = np.zeros(B, np.float32)
    flat_sel = sel.reshape(-1)
    flat_valid = valid.reshape(-1)
    out[flat_sel[flat_valid]] = qv[flat_valid]
    out += b3[idx]
    return out.astype(np.float32)
